# revision 1
# baseline (speedup 1.0000x reference)
"""Cross-attention Trainium2 kernel (8 NeuronCores, batch-data-parallel).

Computes, per batch element b:
    q = x[b] @ Wq            [S, DK]
    k = y[b] @ Wk            [S, DK]
    v = y[b] @ Wv            [S, E]
    p = exp((q @ k.T) / sqrt(E))        (no max-subtraction: logits ~ N(0, .25))
    out[b] = (p @ v) / rowsum(p) + x[b]

Layout strategy (per core, BL=2 batches):
  - Everything on TensorE is bf16 with fp32 PSUM accumulation.
  - Activations are transposed on-chip (cast-DMA fp32->bf16 into a DRAM
    bounce, then xbar DMA-transpose into SBUF) so the contraction dim of
    every matmul sits on partitions:
        xT, yT : [C, S]     qT = Wq.T @ xT : [DK, S]   kT : [DK, S]
        v  = yT.T @ Wv : [S_kv, E]  (natural layout)
        sT = kT.T @ qT : [S_kv, S_q]   (scoresT; softmax axis = partitions)
        pT = exp(sT/32)                (stationary of the AV matmul)
        out = pT.T @ [v | 1]           (ones column yields rowsum(p) free)
  - Epilogue fuses (psum * 1/rowsum) + x in one DVE scalar_tensor_tensor.
"""

import math

import numpy as np

# Full-problem constants (hardcoded per the harness contract).
B_FULL = 16
N_CORES = 8
S_Q = 2048
S_KV = 2048
C_DIM = 1024  # input feature dim (contraction of the projections)
DK = 256  # q/k head dim
E_DIM = 1024  # v / output dim
P = 128


class CFG:
    def __init__(self, bl, sq, skv, c, dk, e, s_block=None, n_free=512):
        assert sq % P == 0 and skv % P == 0 and c % P == 0 and dk % P == 0
        self.bl = bl  # batches per core
        self.sq = sq
        self.skv = skv
        self.c = c
        self.dk = dk
        self.e = e
        self.s_block = s_block or min(1024, sq)  # query cols processed per wave
        assert sq % self.s_block == 0
        self.n_free = n_free  # moving-operand free-dim per matmul
        self.scale = 1.0 / math.sqrt(e)


def _chunks(total, size):
    out = []
    o = 0
    while o < total:
        out.append((o, min(size, total - o)))
        o += size
    return out


def emit_cross_attention(tc, outs, ins, cfg):
    """Emit the kernel into TileContext `tc`.

    outs/ins are dicts of DRAM APs: ins = x, y, Wq, Wk, Wv ; outs = out.
    x/y/out: [bl, sq|skv, c|e] fp32. Weights: [c, dk|e] fp32.
    """
    import concourse.mybir as mybir
    from concourse.mybir import ActivationFunctionType as AF
    from concourse.mybir import AluOpType as ALU
    from concourse.tile_rust import add_dep_helper

    nc = tc.nc
    bf16 = mybir.dt.bfloat16
    f32 = mybir.dt.float32

    x, y, Wq, Wk, Wv = ins["x"], ins["y"], ins["Wq"], ins["Wk"], ins["Wv"]
    out = outs["out"]

    n_ct = cfg.c // P  # contraction tiles of the projections
    n_tt = cfg.skv // P  # key/value tiles (AV contraction)
    n_dt = cfg.dk // P  # qk-dim tiles (score contraction)
    s_waves = _chunks(cfg.sq, cfg.s_block)

    # DRAM bounce buffers for the bf16 copies of x and y (per local batch).
    xb = nc.dram_tensor("xb16", [cfg.bl, cfg.sq, cfg.c], bf16).ap()
    yb = nc.dram_tensor("yb16", [cfg.bl, cfg.skv, cfg.c], bf16).ap()

    pool = tc.alloc_tile_pool(name="main", bufs=1)
    ps_mm = tc.alloc_tile_pool(name="ps_mm", bufs=2, space="PSUM")
    ps_av = tc.alloc_tile_pool(name="ps_av", bufs=2, space="PSUM")

    # Measured DMA facts this layout is built on:
    #   - SWDGE D2D cast runs at ~360 GB/s payload.
    #   - xbar transposes cost ~1us fixed + ~400 GB/s; they only exist on
    #     one ring (concurrent transposes on both HWDGE rings corrupt), and
    #     Tile serializes every transpose group against ALL in-flight DMAs.
    #     So the global stream alternates copy-windows and transpose-windows,
    #     ordered here so each window's data is needed just after it closes.
    #   - SWDGE queue: casts only; sync ring: transposes only; scalar ring:
    #     weights / residual / output plain DMAs.
    half = cfg.skv // 2 if cfg.skv >= 1024 else cfg.skv
    y0_chunks = _chunks(cfg.skv, half)

    wq_sb = []
    wk_sb = []
    wv_sb = []
    for kc in range(n_ct):
        wq_t = pool.tile([P, cfg.dk], bf16, tag=f"wq{kc}", name=f"wq{kc}")
        wk_t = pool.tile([P, cfg.dk], bf16, tag=f"wk{kc}", name=f"wk{kc}")
        wv_t = pool.tile([P, cfg.e], bf16, tag=f"wv{kc}", name=f"wv{kc}")
        wq_sb.append(wq_t)
        wk_sb.append(wk_t)
        wv_sb.append(wv_t)

    def load_weight(which, w_dram, w_tiles, wdim, kc):
        w_f = pool.tile([P, cfg.e], f32, tag="wstage", bufs=2, name=f"wf{which}{kc}")
        nc.scalar.dma_start(out=w_f[:, :wdim], in_=w_dram[kc * P : (kc + 1) * P, :])
        nc.vector.tensor_copy(w_tiles[kc][:], w_f[:, :wdim])

    # copy-window 0: y0 first half cast (SWDGE) + wk loads (scalar ring)
    nc.gpsimd.dma_start(out=yb[0][0:half, :], in_=y[0][0:half, :])
    for kc in range(n_ct):
        load_weight("k", Wk, wk_sb, cfg.dk, kc)

    ones_col = pool.tile([P, 1], bf16, tag="ones", name="ones")
    nc.gpsimd.memset(ones_col[:], 1.0)

    allT = {}
    for b in range(cfg.bl):
        yT = []
        xT = []
        for kc in range(n_ct):
            yT_t = pool.tile([P, cfg.skv], bf16, tag="actT", bufs=2 * n_ct, name=f"yT{kc}")
            yT.append(yT_t)
        for kc in range(n_ct):
            xT_t = pool.tile([P, cfg.sq], bf16, tag="actT", bufs=2 * n_ct, name=f"xT{kc}")
            xT.append(xT_t)
        allT[b] = (yT, xT)

    def transpose_group(b, which, ro, rn):
        srcb = yb if which == "y" else xb
        dst = allT[b][0] if which == "y" else allT[b][1]
        last = None
        for kc in range(n_ct):
            last = nc.sync.dma_start(
                out=dst[kc][:, ro : ro + rn],
                in_=srcb[b][ro : ro + rn, kc * P : (kc + 1) * P],
                transpose=True,
            )
        return last

    def pace(waiter, dependee):
        # Real semaphore edge: keeps the next copy-window out of flight until
        # the previous transpose-window drains (Tile serializes any transpose
        # against every in-flight copy, so un-paced casts stall transposes).
        if waiter is not None and dependee is not None:
            add_dep_helper(waiter.ins, dependee.ins, sync=True, reason="pace dma windows")

    # transpose-window: yT(b0) first half
    tg = transpose_group(0, "y", 0, half)
    # copy-window: y0 second half + wv, wq loads
    if half < cfg.skv:
        c = nc.gpsimd.dma_start(out=yb[0][half:, :], in_=y[0][half:, :])
        pace(c, tg)
    for kc in range(n_ct):
        load_weight("v", Wv, wv_sb, cfg.e, kc)
    for kc in range(n_ct):
        load_weight("q", Wq, wq_sb, cfg.dk, kc)
    if half < cfg.skv:
        tg = transpose_group(0, "y", half, cfg.skv - half)
    # copy-window: x0 cast; then xT(b0) transposes
    c = nc.gpsimd.dma_start(out=xb[0][:], in_=x[0][:])
    pace(c, tg)
    tg = transpose_group(0, "x", 0, cfg.sq)
    allT["last_tg"] = tg
    # b1 chains are emitted inside the batch loop below (their windows land
    # under b0's scores/AV compute).

    for b in range(cfg.bl):
        yT, xT = allT[b]
        if b > 0:
            c = nc.gpsimd.dma_start(out=yb[b][:], in_=y[b][:])
            pace(c, allT["last_tg"])
            tg = transpose_group(b, "y", 0, cfg.skv)
            c = nc.gpsimd.dma_start(out=xb[b][:], in_=x[b][:])
            pace(c, tg)
            pace(c, allT.get(f"wave_end_{b - 1}_0"))
            tg = transpose_group(b, "x", 0, cfg.sq)
            allT["last_tg"] = tg

        # --- projections: kT/v aligned to the y halves, then qT ------------
        kT = []
        qT = []
        for md in range(n_dt):
            kT_t = pool.tile([P, cfg.skv], bf16, tag=f"kT{md}", name=f"kT{md}")
            qT_t = pool.tile([P, cfg.sq], bf16, tag=f"qT{md}", name=f"qT{md}")
            kT.append(kT_t)
            qT.append(qT_t)
        v_sb = [
            pool.tile([P, cfg.e], bf16, tag="v", bufs=n_tt, name=f"v{mt}")
            for mt in range(n_tt)
        ]

        for ro, rn in y0_chunks:
            for no, nn_ in _chunks(rn, cfg.n_free):
                for md in range(n_dt):
                    ps = ps_mm.tile([P, cfg.n_free], f32, tag="mm", name="ps_p")
                    for kc in range(n_ct):
                        nc.tensor.matmul(
                            ps[:, :nn_],
                            wk_sb[kc][:, md * P : (md + 1) * P],
                            yT[kc][:, ro + no : ro + no + nn_],
                            start=(kc == 0),
                            stop=(kc == n_ct - 1),
                        )
                    nc.scalar.activation(
                        kT[md][:, ro + no : ro + no + nn_], ps[:, :nn_], AF.Copy
                    )
            for mt in range(ro // P, (ro + rn) // P):
                v_t = v_sb[mt]
                for no, nn_ in _chunks(cfg.e, cfg.n_free):
                    ps = ps_mm.tile([P, cfg.n_free], f32, tag="mm", name="ps_v")
                    for kc in range(n_ct):
                        nc.tensor.matmul(
                            ps[:, :nn_],
                            yT[kc][:, mt * P : (mt + 1) * P],
                            wv_sb[kc][:, no : no + nn_],
                            start=(kc == 0),
                            stop=(kc == n_ct - 1),
                        )
                    nc.scalar.activation(v_t[:, no : no + nn_], ps[:, :nn_], AF.Copy)
        for no, nn_ in _chunks(cfg.sq, cfg.n_free):
            for md in range(n_dt):
                ps = ps_mm.tile([P, cfg.n_free], f32, tag="mm", name="ps_q")
                for kc in range(n_ct):
                    nc.tensor.matmul(
                        ps[:, :nn_],
                        wq_sb[kc][:, md * P : (md + 1) * P],
                        xT[kc][:, no : no + nn_],
                        start=(kc == 0),
                        stop=(kc == n_ct - 1),
                    )
                nc.scalar.activation(qT[md][:, no : no + nn_], ps[:, :nn_], AF.Copy)

        # --- attention, one wave of s_block query columns at a time --------
        for wo, wn in s_waves:
            # scoresT + exp: pT[t, s_block]
            pT = []
            for t in range(n_tt):
                pT_t = pool.tile([P, cfg.s_block], bf16, tag="pT", bufs=n_tt, name=f"pT{t}")
                for no, nn_ in _chunks(wn, cfg.n_free):
                    ps = ps_mm.tile([P, cfg.n_free], f32, tag="mm", name="ps_s")
                    for kd in range(n_dt):
                        nc.tensor.matmul(
                            ps[:, :nn_],
                            kT[kd][:, t * P : (t + 1) * P],
                            qT[kd][:, wo + no : wo + no + nn_],
                            start=(kd == 0),
                            stop=(kd == n_dt - 1),
                        )
                    nc.scalar.activation(
                        pT_t[:, no : no + nn_], ps[:, :nn_], AF.Exp, scale=cfg.scale
                    )
                pT.append(pT_t)

            # AV + rowsum + epilogue, per 128-row block of queries
            for mh in range(wn // P):
                sm = wo + mh * P  # global query row offset
                ps_e = ps_av.tile([P, cfg.e], f32, tag="av_e", name="ps_e")
                ps_sum = ps_av.tile([P, 1], f32, tag="av_s", name="ps_sum")
                e_chunks = _chunks(cfg.e, cfg.n_free)
                for t in range(n_tt):
                    lhsT = pT[t][:, mh * P : (mh + 1) * P]
                    for no, nn_ in e_chunks:
                        nc.tensor.matmul(
                            ps_e[:, no : no + nn_],
                            lhsT,
                            v_sb[t][:, no : no + nn_],
                            start=(t == 0),
                            stop=(t == n_tt - 1),
                        )
                    nc.tensor.matmul(
                        ps_sum[:],
                        lhsT,
                        ones_col[:],
                        start=(t == 0),
                        stop=(t == n_tt - 1),
                    )
                recip = pool.tile([P, 1], f32, tag="recip", bufs=4, name="recip")
                nc.vector.reciprocal(recip[:], ps_sum[:])
                xres = pool.tile([P, cfg.e], f32, tag="xres", bufs=3, name="xres")
                nc.scalar.dma_start(out=xres[:], in_=x[b][sm : sm + P, :])
                out_t = pool.tile([P, cfg.e], f32, tag="out_t", bufs=4, name="out_t")
                nc.vector.scalar_tensor_tensor(
                    out_t[:], ps_e[:], recip[:], xres[:], ALU.mult, ALU.add
                )
                st = nc.scalar.dma_start(out=out[b][sm : sm + P, :], in_=out_t[:])
                allT[f"wave_end_{b}_{wo}"] = st

    ps_av.release()
    ps_mm.release()
    pool.release()


def make_tile_kernel(cfg):
    """Adapter with the (tc, outs, ins) signature used by run_kernel/test.py."""

    def k(tc, outs, ins):
        emit_cross_attention(tc, outs, ins, cfg)

    return k


def _build(cfg):
    import concourse.bacc as bacc
    import concourse.mybir as mybir
    import concourse.tile as tile

    f32 = mybir.dt.float32
    nc = bacc.Bacc(
        "TRN2",
        target_bir_lowering=False,
        debug=False,
        enable_asserts=False,
        num_devices=N_CORES,
    )
    ins = {
        "x": nc.dram_tensor("x", [cfg.bl, cfg.sq, cfg.c], f32, kind="ExternalInput").ap(),
        "y": nc.dram_tensor("y", [cfg.bl, cfg.skv, cfg.c], f32, kind="ExternalInput").ap(),
        "Wq": nc.dram_tensor("Wq", [cfg.c, cfg.dk], f32, kind="ExternalInput").ap(),
        "Wk": nc.dram_tensor("Wk", [cfg.c, cfg.dk], f32, kind="ExternalInput").ap(),
        "Wv": nc.dram_tensor("Wv", [cfg.c, cfg.e], f32, kind="ExternalInput").ap(),
    }
    outs = {
        "out": nc.dram_tensor("out", [cfg.bl, cfg.sq, cfg.e], f32, kind="ExternalOutput").ap()
    }
    with tile.TileContext(nc) as tc:
        emit_cross_attention(tc, outs, ins, cfg)
    nc.compile()
    return nc


_CACHED = {}


def run_on_cores(x, y, Wq, Wk, Wv, trace=False):
    from concourse import bass_utils

    cfg = CFG(B_FULL // N_CORES, S_Q, S_KV, C_DIM, DK, E_DIM)
    key = "full"
    if key not in _CACHED:
        _CACHED[key] = _build(cfg)
    nc = _CACHED[key]

    bl = cfg.bl
    in_maps = [
        {
            "x": np.ascontiguousarray(x[i * bl : (i + 1) * bl]),
            "y": np.ascontiguousarray(y[i * bl : (i + 1) * bl]),
            "Wq": Wq,
            "Wk": Wk,
            "Wv": Wv,
        }
        for i in range(N_CORES)
    ]
    res = bass_utils.run_bass_kernel_spmd(
        nc, in_maps, core_ids=list(range(N_CORES)), trace=trace
    )
    out = np.concatenate([r["out"] for r in res.results], axis=0)
    return out, res


def kernel(x, y, Wq, Wk, Wv):
    x = np.asarray(x, dtype=np.float32)
    y = np.asarray(y, dtype=np.float32)
    Wq = np.asarray(Wq, dtype=np.float32)
    Wk = np.asarray(Wk, dtype=np.float32)
    Wv = np.asarray(Wv, dtype=np.float32)
    out, _ = run_on_cores(x, y, Wq, Wk, Wv, trace=False)
    return out



# revision 3
# speedup vs baseline: 1.4049x; 1.4049x over previous
"""Cross-attention Trainium2 kernel (8 NeuronCores, batch-data-parallel).

Computes, per batch element b:
    q = x[b] @ Wq            [S, DK]
    k = y[b] @ Wk            [S, DK]
    v = y[b] @ Wv            [S, E]
    p = exp((q @ k.T) / sqrt(E))        (no max-subtraction: logits ~ N(0, .25))
    out[b] = (p @ v) / rowsum(p) + x[b]

All matmuls run in fp8e4 DoubleRow mode (K=256 per matmul, 2x bf16 rate).
Weights are pre-scaled by 16 on-chip so their values sit in fp8's normal
range; the extra 16*16 factor on scores folds into the exp scale and the
16 on v folds into the rowsum (ones column holds 16.0).

Data movement (per core, BL=2 batches):
  - SWDGE D2D cast fp32 -> fp8 into a DRAM bounce ([S, C] fp8).
  - The bounce is bitcast to bf16 pairs [S, C/2] and xbar DMA-transposed
    into SBUF: tiles xT[t4] = [128 chan-pairs, S] where partition p of
    tile t4 holds channels (256*t4 + 2p, +1) interleaved along the free
    dim.  These serve directly as DoubleRow *moving* operands
    ([128, 2, N] with strides (1, 2)).
  - DoubleRow *stationary* operands must be pair-blocked (LDWEIGHTS
    rejects stride-1 pair dim), so yT is additionally deinterleaved on
    DVE into yT_blk [128, 2, S] for use as the V-projection stationary.
  - Weight tiles load as [128, 2, M] fp32 (rows 256*t4+2p+j) and cast
    to fp8 with scale=16 on ScalarE.

Matmul structure (contraction 256 per DR matmul):
    kT = wk8[t4].T *dr* yT[t4]   [2, 128dk, S_kv]   (stat=wk, mov=yT int)
    v  = yT_blk[t4].T *dr* wv8[t4]  [S_kv, E]       (stat=yT blk, mov=wv)
    qT = wq8[t4].T *dr* xT[t4]   [2, 128dk, S_q]    (stat=wq, mov=xT int)
    sT = kT.T *dr* qT            [t, s]  per (key-tile, 512-chunk)
    pT = exp(sT * scale)         fp8, blocked per key-pair [128, 2, s_blk]
    out = pT.T *dr* [v | 16]     accumulated over 8 key-pairs
Epilogue fuses (psum * 1/rowsum) + x in one DVE scalar_tensor_tensor.

Scores for wave w+1 are emitted interleaved into wave w's AV groups so
ScalarE exp never stalls the PE.
"""

import math

import numpy as np

# Full-problem constants (hardcoded per the harness contract).
B_FULL = 16
N_CORES = 8
S_Q = 2048
S_KV = 2048
C_DIM = 1024  # input feature dim (contraction of the projections)
DK = 256  # q/k head dim
E_DIM = 1024  # v / output dim
P = 128
WSCALE = 16.0  # fp8 range pre-scale applied to all weights


class CFG:
    def __init__(self, bl, sq, skv, c, dk, e, s_block=1024):
        assert sq % P == 0 and skv % P == 0 and c % 256 == 0 and dk == 256
        self.bl = bl  # batches per core
        self.sq = sq
        self.skv = skv
        self.c = c
        self.dk = dk
        self.e = e
        self.s_block = min(s_block, sq)  # query cols per wave
        assert sq % self.s_block == 0
        assert self.s_block % 512 == 0
        # exp( (q.k) / sqrt(E) ) with both q and k carrying WSCALE
        self.scale = 1.0 / (math.sqrt(e) * WSCALE * WSCALE)


def _chunks(total, size):
    out = []
    o = 0
    while o < total:
        out.append((o, min(size, total - o)))
        o += size
    return out


def emit_cross_attention(tc, outs, ins, cfg):
    """Emit the kernel into TileContext `tc`.

    ins = x, y, Wq, Wk, Wv ; outs = out.
    x/y/out: [bl, sq|skv, c|e] fp32. Weights: [c, dk|e] fp32.
    """
    import concourse.mybir as mybir
    from concourse.mybir import ActivationFunctionType as AF
    from concourse.mybir import AluOpType as ALU
    from concourse.mybir import MatmulPerfMode
    from concourse.tile_rust import add_dep_helper

    nc = tc.nc
    bf16 = mybir.dt.bfloat16
    fp8 = mybir.dt.float8e4
    f32 = mybir.dt.float32
    DR = MatmulPerfMode.DoubleRow

    x, y, Wq, Wk, Wv = ins["x"], ins["y"], ins["Wq"], ins["Wk"], ins["Wv"]
    out = outs["out"]

    nt4 = cfg.c // 256  # channel pair-tiles (256 channels each)
    nt = cfg.skv // P  # key tiles
    nkp = nt // 2  # key pair-tiles
    nd = cfg.dk // P  # dk tiles (2)
    nec = cfg.e // 512  # e chunks
    waves = _chunks(cfg.sq, cfg.s_block)
    n_mh_w = cfg.s_block // P  # query tiles per wave

    # DRAM bounce buffers for the fp8 copies of x and y.
    xb = nc.dram_tensor("xb8", [cfg.bl, cfg.sq, cfg.c], fp8).ap()
    yb = nc.dram_tensor("yb8", [cfg.bl, cfg.skv, cfg.c], fp8).ap()
    xb16 = xb.bitcast(bf16)  # [bl, sq, c/2]
    yb16 = yb.bitcast(bf16)

    pool = tc.alloc_tile_pool(name="main", bufs=1)
    ps_mm = tc.alloc_tile_pool(name="ps_mm", bufs=3, space="PSUM")
    ps_av = tc.alloc_tile_pool(name="ps_av", bufs=2, space="PSUM")
    ps_sm = tc.alloc_tile_pool(name="ps_sm", bufs=1, space="PSUM")

    # ---- weights: [128, 2, M] fp32 staging -> fp8 * WSCALE ---------------
    wq8, wk8, wv8 = [], [], []
    w_loads = []  # (which, t4) emission plan interleaved with y0 casts

    def load_weight(w_dram, wdim, t4, name):
        stage = pool.tile([P, 2, cfg.e], f32, tag="wstage", bufs=1,
                          name=f"ws{name}{t4}")
        src = w_dram[256 * t4:256 * (t4 + 1), :].rearrange(
            "(p j) m -> p j m", j=2)
        nc.scalar.dma_start(out=stage[:, :, :wdim], in_=src)
        w8 = pool.tile([P, 2, wdim], fp8, tag=f"w8{name}{t4}",
                       name=f"w8{name}{t4}")
        nc.scalar.activation(w8[:], stage[:, :, :wdim], AF.Copy, scale=WSCALE)
        return w8

    for t4 in range(nt4):
        wk8.append(load_weight(Wk, cfg.dk, t4, "k"))
    for t4 in range(nt4):
        wv8.append(load_weight(Wv, cfg.e, t4, "v"))
    for t4 in range(nt4):
        wq8.append(load_weight(Wq, cfg.dk, t4, "q"))

    ones16 = pool.tile([P, 2, 1], fp8, tag="ones", name="ones")
    nc.gpsimd.memset(ones16[:], WSCALE)

    # ---- activation transpose machinery ---------------------------------
    # Measured DMA facts this layout is built on:
    #   - SWDGE D2D cast ~360 GB/s payload; casts go on the SWDGE queue.
    #   - xbar transposes (sync ring only) cost ~1us fixed + ~400 GB/s and
    #     Tile serializes every transpose group against ALL in-flight DMAs,
    #     so the stream alternates cast-windows and transpose-windows.
    half = cfg.skv // 2

    def make_T_tiles(which):
        # bf16-typed [128, S] tiles; fp8 pair-interleaved views for matmul
        s = cfg.sq if which == "x" else cfg.skv
        tiles = []
        for t4 in range(nt4):
            t = pool.tile([P, s], bf16, tag=f"{which}T", bufs=2 * nt4,
                          name=f"{which}T{t4}")
            tiles.append(t)
        return tiles

    def int_view(t):
        # [128, S, 2] fp8 view of a bf16 transpose tile
        return t[:].bitcast(fp8).rearrange("p (s j) -> p s j", j=2)

    allT = {}

    def transpose_group(b, which, ro, rn):
        srcb = yb16 if which == "y" else xb16
        dst = allT[(b, which)]
        last = None
        for t4 in range(nt4):
            last = nc.sync.dma_start(
                out=dst[t4][:, ro:ro + rn],
                in_=srcb[b][ro:ro + rn, t4 * P:(t4 + 1) * P],
                transpose=True,
            )
        return last

    def deinterleave_y(b, ro, rn):
        yT = allT[(b, "y")]
        blk = allT[(b, "yblk")]
        for t4 in range(nt4):
            nc.vector.tensor_copy(
                blk[t4][:, :, ro:ro + rn],
                int_view(yT[t4])[:, ro:ro + rn, :].transpose([0, 2, 1]),
            )

    def pace(waiter, dependee):
        # Real semaphore edge: keeps the next cast-window out of flight
        # until the previous transpose-window drains.
        if waiter is not None and dependee is not None:
            add_dep_helper(waiter.ins, dependee.ins, sync=True,
                           reason="pace dma windows")

    for b in range(cfg.bl):
        allT[(b, "x")] = make_T_tiles("x")
        allT[(b, "y")] = make_T_tiles("y")
        allT[(b, "yblk")] = [
            pool.tile([P, 2, cfg.skv], fp8, tag="yblk", bufs=2 * nt4,
                      name=f"yblk{t4}")
            for t4 in range(nt4)
        ]

    # b0 input stream: y halves (kT/v can start on half 1), then x.
    nc.gpsimd.dma_start(out=yb[0][0:half, :], in_=y[0][0:half, :])
    tg = transpose_group(0, "y", 0, half)
    deinterleave_y(0, 0, half)
    c = nc.gpsimd.dma_start(out=yb[0][half:, :], in_=y[0][half:, :])
    pace(c, tg)
    tg = transpose_group(0, "y", half, cfg.skv - half)
    deinterleave_y(0, half, cfg.skv - half)
    c = nc.gpsimd.dma_start(out=xb[0][:], in_=x[0][:])
    pace(c, tg)
    tg = transpose_group(0, "x", 0, cfg.sq)
    allT["last_tg"] = tg

    for b in range(cfg.bl):
        yT = allT[(b, "y")]
        xT = allT[(b, "x")]
        yblk = allT[(b, "yblk")]
        if b > 0:
            c = nc.gpsimd.dma_start(out=yb[b][:], in_=y[b][:])
            pace(c, allT["last_tg"])
            tg = transpose_group(b, "y", 0, cfg.skv)
            deinterleave_y(b, 0, cfg.skv)
            c = nc.gpsimd.dma_start(out=xb[b][:], in_=x[b][:])
            pace(c, tg)
            pace(c, allT.get(f"wave_end_{b - 1}_0"))
            tg = transpose_group(b, "x", 0, cfg.sq)
            allT["last_tg"] = tg

        # ---- projections -------------------------------------------------
        # kT8[p, md, key] = (16k)[key, 128*md+p];  qT8 likewise.
        kT8 = pool.tile([P, nd, cfg.skv], fp8, tag="kT", bufs=2, name="kT")
        qT8 = pool.tile([P, nd, cfg.sq], fp8, tag="qT", bufs=2, name="qT")
        # v8[p, t, e] = (16v)[128*t + p, e]
        v8 = pool.tile([P, nt, cfg.e], fp8, tag="v8", bufs=1, name="v8")

        # kT: stationary wk8[t4][:, :, md*128:...], moving yT int view.
        # Loop so each stationary serves 2 chunk-matmuls (LDW amortize);
        # psum live = 2 per (md, cpair).
        for md in range(nd):
            for co, cn in _chunks(cfg.skv, 1024):
                pss = [ps_mm.tile([P, 512], f32, tag="mm", name="ps_k")
                       for _ in range(cn // 512)]
                for t4 in range(nt4):
                    stat = wk8[t4][:, :, md * P:(md + 1) * P]
                    for ci, ps in enumerate(pss):
                        mov = int_view(yT[t4])[
                            :, co + 512 * ci:co + 512 * (ci + 1), :
                        ].transpose([0, 2, 1])
                        nc.tensor.matmul(ps[:], stat, mov,
                                         start=(t4 == 0), stop=(t4 == nt4 - 1),
                                         perf_mode=DR)
                for ci, ps in enumerate(pss):
                    nc.scalar.activation(
                        kT8[:, md, co + 512 * ci:co + 512 * (ci + 1)],
                        ps[:], AF.Copy)

        # v: stationary yblk slice per (t4, t); moving wv8[t4] e-chunks.
        for t in range(nt):
            ps_v = ps_av.tile([P, cfg.e], f32, tag="av", name="ps_v")
            for t4 in range(nt4):
                stat = yblk[t4][:, :, t * P:(t + 1) * P]
                for ec in range(nec):
                    nc.tensor.matmul(ps_v[:, 512 * ec:512 * (ec + 1)],
                                     stat, wv8[t4][:, :, 512 * ec:512 * (ec + 1)],
                                     start=(t4 == 0), stop=(t4 == nt4 - 1),
                                     perf_mode=DR)
            nc.vector.tensor_copy(v8[:, t, :], ps_v[:])

        # qT
        for md in range(nd):
            for co, cn in _chunks(cfg.sq, 1024):
                pss = [ps_mm.tile([P, 512], f32, tag="mm", name="ps_q")
                       for _ in range(cn // 512)]
                for t4 in range(nt4):
                    stat = wq8[t4][:, :, md * P:(md + 1) * P]
                    for ci, ps in enumerate(pss):
                        mov = int_view(xT[t4])[
                            :, co + 512 * ci:co + 512 * (ci + 1), :
                        ].transpose([0, 2, 1])
                        nc.tensor.matmul(ps[:], stat, mov,
                                         start=(t4 == 0), stop=(t4 == nt4 - 1),
                                         perf_mode=DR)
                for ci, ps in enumerate(pss):
                    nc.scalar.activation(
                        qT8[:, md, co + 512 * ci:co + 512 * (ci + 1)],
                        ps[:], AF.Copy)

        # ---- attention waves --------------------------------------------
        # pT[kp][ki, ko, s] = p[key = 128*(2kp+ko) + ki, wave_off + s]
        def emit_scores(wo, t, pT_w):
            # one key-tile's scores for the whole wave (s_block cols)
            stat = kT8[:, :, t * P:(t + 1) * P]
            n_ch = cfg.s_block // 512
            pss = [ps_mm.tile([P, 512], f32, tag="mm", name="ps_s")
                   for _ in range(n_ch)]
            for ci, ps in enumerate(pss):
                mov = qT8[:, :, wo + 512 * ci:wo + 512 * (ci + 1)]
                nc.tensor.matmul(ps[:], stat, mov, start=True, stop=True,
                                 perf_mode=DR)
            for ci, ps in enumerate(pss):
                nc.scalar.activation(
                    pT_w[t // 2][:, t % 2, 512 * ci:512 * (ci + 1)],
                    ps[:], AF.Exp, scale=cfg.scale)

        def make_pT():
            return [
                pool.tile([P, 2, cfg.s_block], fp8, tag="pT", bufs=2 * nkp,
                          name=f"pT{kp}")
                for kp in range(nkp)
            ]

        pT_cur = None
        for wi, (wo, wn) in enumerate(waves):
            if wi == 0:
                # no previous AV to hide behind: emit wave-0 scores directly
                pT_cur = make_pT()
                for t in range(nt):
                    emit_scores(wo, t, pT_cur)
            pT_next = make_pT() if wi + 1 < len(waves) else None

            # AV + rowsum + epilogue per 128-query tile, with next wave's
            # scores interleaved (nt key-tiles spread over n_mh_w groups).
            spt = nt // n_mh_w if pT_next is not None else 0
            for mh in range(n_mh_w):
                sm = wo + mh * P
                ps_e = ps_av.tile([P, cfg.e], f32, tag="av", name="ps_e")
                ps_sum = ps_sm.tile([P, 1], f32, tag="sum", name="ps_sum")
                for kp in range(nkp):
                    stat = pT_cur[kp][:, :, mh * P:(mh + 1) * P]
                    for ec in range(nec):
                        nc.tensor.matmul(
                            ps_e[:, 512 * ec:512 * (ec + 1)],
                            stat, v8[:, 2 * kp:2 * kp + 2,
                                     512 * ec:512 * (ec + 1)],
                            start=(kp == 0), stop=(kp == nkp - 1),
                            perf_mode=DR)
                    nc.tensor.matmul(ps_sum[:], stat, ones16[:],
                                     start=(kp == 0), stop=(kp == nkp - 1),
                                     perf_mode=DR)
                # interleave next wave's scores into the PE stream
                for t in range(mh * spt, (mh + 1) * spt):
                    emit_scores(wo + cfg.s_block, t, pT_next)

                recip = pool.tile([P, 1], f32, tag="recip", bufs=4,
                                  name="recip")
                nc.vector.reciprocal(recip[:], ps_sum[:])
                xres = pool.tile([P, cfg.e], f32, tag="xres", bufs=2,
                                 name="xres")
                nc.scalar.dma_start(out=xres[:], in_=x[b][sm:sm + P, :])
                out_t = pool.tile([P, cfg.e], f32, tag="out_t", bufs=3,
                                  name="out_t")
                nc.vector.scalar_tensor_tensor(
                    out_t[:], ps_e[:], recip[:], xres[:], ALU.mult, ALU.add)
                st = nc.scalar.dma_start(out=out[b][sm:sm + P, :], in_=out_t[:])
                allT[f"wave_end_{b}_{wo}"] = st
            pT_cur = pT_next

    ps_sm.release()
    ps_av.release()
    ps_mm.release()
    pool.release()


def make_tile_kernel(cfg):
    """Adapter with the (tc, outs, ins) signature used by run_kernel/test.py."""

    def k(tc, outs, ins):
        emit_cross_attention(tc, outs, ins, cfg)

    return k


def _build(cfg):
    import concourse.bacc as bacc
    import concourse.mybir as mybir
    import concourse.tile as tile

    f32 = mybir.dt.float32
    nc = bacc.Bacc(
        "TRN2",
        target_bir_lowering=False,
        debug=False,
        enable_asserts=False,
        num_devices=N_CORES,
    )
    ins = {
        "x": nc.dram_tensor("x", [cfg.bl, cfg.sq, cfg.c], f32, kind="ExternalInput").ap(),
        "y": nc.dram_tensor("y", [cfg.bl, cfg.skv, cfg.c], f32, kind="ExternalInput").ap(),
        "Wq": nc.dram_tensor("Wq", [cfg.c, cfg.dk], f32, kind="ExternalInput").ap(),
        "Wk": nc.dram_tensor("Wk", [cfg.c, cfg.dk], f32, kind="ExternalInput").ap(),
        "Wv": nc.dram_tensor("Wv", [cfg.c, cfg.e], f32, kind="ExternalInput").ap(),
    }
    outs = {
        "out": nc.dram_tensor("out", [cfg.bl, cfg.sq, cfg.e], f32, kind="ExternalOutput").ap()
    }
    with tile.TileContext(nc) as tc:
        emit_cross_attention(tc, outs, ins, cfg)
    nc.compile()
    return nc


_CACHED = {}


def run_on_cores(x, y, Wq, Wk, Wv, trace=False):
    from concourse import bass_utils

    cfg = CFG(B_FULL // N_CORES, S_Q, S_KV, C_DIM, DK, E_DIM)
    key = "full"
    if key not in _CACHED:
        _CACHED[key] = _build(cfg)
    nc = _CACHED[key]

    bl = cfg.bl
    in_maps = [
        {
            "x": np.ascontiguousarray(x[i * bl : (i + 1) * bl]),
            "y": np.ascontiguousarray(y[i * bl : (i + 1) * bl]),
            "Wq": Wq,
            "Wk": Wk,
            "Wv": Wv,
        }
        for i in range(N_CORES)
    ]
    res = bass_utils.run_bass_kernel_spmd(
        nc, in_maps, core_ids=list(range(N_CORES)), trace=trace
    )
    out = np.concatenate([r["out"] for r in res.results], axis=0)
    return out, res


def kernel(x, y, Wq, Wk, Wv):
    x = np.asarray(x, dtype=np.float32)
    y = np.asarray(y, dtype=np.float32)
    Wq = np.asarray(Wq, dtype=np.float32)
    Wk = np.asarray(Wk, dtype=np.float32)
    Wv = np.asarray(Wv, dtype=np.float32)
    out, _ = run_on_cores(x, y, Wq, Wk, Wv, trace=False)
    return out
